# revision 30
# baseline (speedup 1.0000x reference)
"""Causal self-attention on 8 Trainium2 NeuronCores, head-sharded tensor parallel.

Contract: kernel(**inputs) takes the FULL unsharded inputs (x, W_qkv, b_qkv,
W_proj, b_proj) as numpy arrays and returns the FULL [B, T, C] float32 output.

Sharding: 16 heads / 8 cores = 2 heads per core. Each core computes qkv for
its heads, causal attention, and a partial output projection
(y_local @ W_proj[head_rows]); the host sums the 8 partials (the
tensor-parallel all-reduce, done at gather time) and adds b_proj.

Per-core kernel design (matmuls in bf16, fp32 accumulate):
- x is pre-transposed on host to xT [C, B*T] (bf16) so the contraction dim C
  is on SBUF partitions for the qkv matmuls.
- Q^T, K^T, V^T are produced in [d, t] layout (both heads packed into 128
  partitions, N=512 matmuls); V is then flipped to natural [t, d] layout by
  PE transposes, with a ones column appended per head ("V-aug") so the PV
  matmul also emits the softmax denominators l as psum row 64.
- Scores are computed transposed, S^T[j, i] = K Q^T, so exp(S^T) = P^T is
  born in the layout the PV matmul needs as rhs (no transposes of P).
- Softmax skips the max-subtraction: logits are ~N(0,1) by construction
  (1/sqrt(D) folded into Wq on host), max |logit| ~ 6, exp is safe in f32.
- Causal structure is exact at 128x128 tile granularity: above-diagonal
  tiles are never computed; diagonal tiles get an additive -87 mask.
- Y^T is normalized before eviction: l row -> columns (K=1 matmuls) ->
  reciprocal -> back to a row (K=128 matmul vs identity) -> gpsimd
  partition-broadcast -> one DVE multiply per head. The projection is then
  a single K=128 matmul per tile with a plain copy eviction.
"""
import sys

sys.path.insert(0, "/opt/trn_rl_repo")

import numpy as np
import ml_dtypes

import concourse.bacc as bacc
import concourse.bass as bass
import concourse.mybir as mybir
import concourse.tile as tile
from concourse import bass_utils

B, T, C, H, D = 2, 2048, 1024, 16, 64
NCORES = 8
HL = H // NCORES          # heads per core = 2
BT = B * T                # 4096
KT = C // 128             # 8 contraction tiles over C
NMC = BT // 1024          # 4 merged (1024-wide) column chunks over B*T
NTB = BT // 128           # 32 t-blocks of 128
NIC = T // 512            # 4 i-chunks per batch
BF16 = mybir.dt.bfloat16
F32 = mybir.dt.float32
AF = mybir.ActivationFunctionType
ALU = mybir.AluOpType
MASK_VAL = -87.0

_compiled = {}
_SKIP_LCHAIN = False
_SKIP_ATTN = False
_SKIP_PROJ = False
_SKIP_VT = False


def _build():
    nc = bacc.Bacc("TRN2", target_bir_lowering=False, debug=False)

    xt_d = nc.dram_tensor("xt", [C, BT], BF16, kind="ExternalInput")
    wq_d = nc.dram_tensor("wq", [C, 128], BF16, kind="ExternalInput")
    wk_d = nc.dram_tensor("wk", [C, 128], BF16, kind="ExternalInput")
    wv_d = nc.dram_tensor("wv", [C, 128], BF16, kind="ExternalInput")
    wp_d = nc.dram_tensor("wp", [128, C], BF16, kind="ExternalInput")
    bq_d = nc.dram_tensor("bq", [128, 1], F32, kind="ExternalInput")
    bk_d = nc.dram_tensor("bk", [128, 1], F32, kind="ExternalInput")
    bvb_d = nc.dram_tensor("bvb", [128, 128], F32, kind="ExternalInput")
    maskbf_d = nc.dram_tensor("maskbf", [128, 128], BF16, kind="ExternalInput")
    idbf_d = nc.dram_tensor("idbf", [128, 128], BF16, kind="ExternalInput")
    out_d = nc.dram_tensor("out", [BT, C], F32, kind="ExternalOutput")
    lsc = nc.dram_tensor("lsc", [B, NIC, 1024], F32)

    with tile.TileContext(nc) as tc:
        consts = tc.alloc_tile_pool(name="consts", bufs=1)
        bigbufs = tc.alloc_tile_pool(name="bigbufs", bufs=1)
        pts = tc.alloc_tile_pool(name="pts", bufs=3)
        lpool = tc.alloc_tile_pool(name="lpool", bufs=2)
        ostage = tc.alloc_tile_pool(name="ostage", bufs=4)
        psum = tc.alloc_tile_pool(name="psum", bufs=1, space="PSUM")

        def ps_s():
            return psum.tile([128, 2, 512], F32, tag="s", bufs=2, name="ps_s")

        # ---- constants ----
        wq_sb = consts.tile([128, KT, 128], BF16)
        wk_sb = consts.tile([128, KT, 128], BF16)
        wv_sb = consts.tile([128, KT, 128], BF16)
        for w_sb, w_d in ((wq_sb, wq_d), (wk_sb, wk_d), (wv_sb, wv_d)):
            nc.sync.dma_start(out=w_sb[:], in_=w_d.ap().rearrange("(k p) m -> p k m", p=128))
        wp_sb = consts.tile([128, C], BF16)
        nc.sync.dma_start(out=wp_sb[:], in_=wp_d[:, :])
        bq_sb = consts.tile([128, 1], F32)
        bk_sb = consts.tile([128, 1], F32)
        bvb_sb = consts.tile([128, 128], F32)
        maskbf_sb = consts.tile([128, 128], BF16)
        idbf_sb = consts.tile([128, 128], BF16)
        nc.sync.dma_start(out=bq_sb[:], in_=bq_d[:, :])
        nc.sync.dma_start(out=bk_sb[:], in_=bk_d[:, :])
        nc.sync.dma_start(out=bvb_sb[:], in_=bvb_d[:, :])
        nc.sync.dma_start(out=maskbf_sb[:], in_=maskbf_d[:, :])
        nc.sync.dma_start(out=idbf_sb[:], in_=idbf_d[:, :])

        # ---- persistent big buffers ----
        xt_sb = bigbufs.tile([128, KT, BT], BF16)       # 8 MB
        for g in range(NMC):
            for k in range(KT):
                nc.sync.dma_start(
                    out=xt_sb[:, k, bass.ts(g, 1024)],
                    in_=xt_d[k * 128:(k + 1) * 128, bass.ts(g, 1024)])
        qT = bigbufs.tile([128, BT], BF16)              # [2h*64 d, t]
        kTt = bigbufs.tile([128, BT], BF16)
        v_sb = bigbufs.tile([128, NTB, 130], BF16)      # [t, tb, 1|Vh0|Vh1|1]
        yt = bigbufs.tile([128, B, T], BF16)            # [2h*64 d, b, t] normalized

        nc.vector.memset(v_sb[:, :, 64:65], 1.0)
        nc.vector.memset(v_sb[:, :, 129:130], 1.0)

        # ---- stage A: Q^T, K^T (d-major, N=512 matmuls) ----
        for w_sb, b_sb, dst in ((wq_sb, bq_sb, qT), (wk_sb, bk_sb, kTt)):
            for mc in range(NMC):
                ps = ps_s()
                for k in range(KT):
                    for half in range(2):
                        nc.tensor.matmul(
                            ps[:, half, :], lhsT=w_sb[:, k, :],
                            rhs=xt_sb[:, k, mc * 1024 + half * 512:mc * 1024 + (half + 1) * 512],
                            start=(k == 0), stop=(k == KT - 1))
                nc.vector.tensor_scalar_add(
                    dst[:, bass.ts(mc, 1024)],
                    ps.rearrange("p h x -> p (h x)"), b_sb[:])

        # V in natural [t, d] layout directly: lhsT = xT tile, rhs = Wv
        for tb in range(NTB):
            pv = psum.tile([128, 128], F32, tag="y", bufs=2)
            for k in range(KT):
                nc.tensor.matmul(pv[:], lhsT=xt_sb[:, k, bass.ts(tb, 128)],
                                 rhs=wv_sb[:, k, :], start=(k == 0), stop=(k == KT - 1))
            nc.vector.tensor_add(v_sb[:, tb, 0:64], pv[:, 0:64], bvb_sb[:, 0:64])
            nc.vector.tensor_add(v_sb[:, tb, 65:129], pv[:, 64:128], bvb_sb[:, 64:128])

        # ---- stage B: attention + projection per batch ----
        for b in range(B):
            for ci in (range(NIC) if b == 0 else range(NIC - 1, -1, -1)):
                y0 = psum.tile([65, 512], F32, tag="y", bufs=2)
                y1 = psum.tile([65, 512], F32, tag="y", bufs=2)
                njb = 4 * (ci + 1)
                tg = b * T + ci * 512
                for jb in ([] if _SKIP_ATTN else range(njb)):
                    sb = max(0, jb - 4 * ci)
                    lo = sb * 128
                    jg = b * T + jb * 128
                    s2 = ps_s()
                    diag = jb >= 4 * ci
                    nc.tensor.matmul(s2[:, 0, lo:512], lhsT=kTt[0:64, jg:jg + 128],
                                     rhs=qT[0:64, tg + lo:tg + 512], start=True,
                                     stop=not diag)
                    nc.tensor.matmul(s2[:, 1, lo:512], lhsT=kTt[64:128, jg:jg + 128],
                                     rhs=qT[64:128, tg + lo:tg + 512], start=True,
                                     stop=not diag)
                    if diag:  # add the causal mask onto the diagonal 128 cols
                        nc.tensor.matmul(s2[:, 0, lo:lo + 128], lhsT=idbf_sb[:],
                                         rhs=maskbf_sb[:], start=False, stop=True)
                        nc.tensor.matmul(s2[:, 1, lo:lo + 128], lhsT=idbf_sb[:],
                                         rhs=maskbf_sb[:], start=False, stop=True)
                    ptb = pts.tile([128, 2, 512], BF16, tag="pt", bufs=4)
                    nc.scalar.activation(ptb[:, 0, lo:512], s2[:, 0, lo:512], AF.Exp)
                    vt = b * 16 + jb
                    nc.tensor.matmul(y0[:, lo:512], lhsT=v_sb[:, vt, 0:65],
                                     rhs=ptb[:, 0, lo:512], start=(jb == 0), stop=(jb == njb - 1))
                    nc.scalar.activation(ptb[:, 1, lo:512], s2[:, 1, lo:512], AF.Exp)
                    nc.tensor.matmul(y1[:, lo:512], lhsT=v_sb[:, vt, 65:130],
                                     rhs=ptb[:, 1, lo:512], start=(jb == 0), stop=(jb == njb - 1))

                # evict unnormalized Y^T immediately (frees the y psums)
                nc.vector.tensor_copy(yt[0:64, b, ci * 512:(ci + 1) * 512],
                                      y0[0:64, :])
                nc.vector.tensor_copy(yt[64:128, b, ci * 512:(ci + 1) * 512],
                                      y1[0:64, :])
                # denominator rows -> columns (DRAM-bounced transpose) -> 1/l
                lr2 = lpool.tile([1, 1024], F32, tag="lr2", bufs=2)
                nc.scalar.copy(lr2[0:1, 0:512], y0[64:65, :])
                nc.scalar.copy(lr2[0:1, 512:1024], y1[64:65, :])
                nc.sync.dma_start(out=lsc[b, ci], in_=lr2[:])
                lcol = lpool.tile([128, 8], F32, tag="lcol", bufs=2)
                for h in range(HL):
                    nc.sync.dma_start(
                        out=lcol[:, h * 4:(h + 1) * 4],
                        in_=lsc[b, ci][h * 512:(h + 1) * 512].rearrange(
                            "(c p) -> p c", p=128))
                linv = lpool.tile([128, 8], F32, tag="linv", bufs=2)
                nc.vector.reciprocal(linv[:], lcol[:])

                # projection for this chunk's 4 t-blocks: per-head K=64
                # row-packed pairs, 1/l applied per head at eviction
                for tb4 in range(4):
                    tgp = ci * 512 + tb4 * 128
                    ot = ostage.tile([128, 1024], F32, tag="ot", bufs=4)
                    for cc in range(2):
                        p0c = psum.tile([128, 512], F32, tag="pj", bufs=2, name="p0c")
                        p1c = psum.tile([128, 512], F32, tag="pj", bufs=2, name="p1c")
                        nc.tensor.matmul(p0c[:], lhsT=yt[0:64, b, tgp:tgp + 128],
                                         rhs=wp_sb[0:64, bass.ts(cc, 512)],
                                         start=True, stop=True)
                        nc.tensor.matmul(p1c[:], lhsT=yt[64:128, b, tgp:tgp + 128],
                                         rhs=wp_sb[64:128, bass.ts(cc, 512)],
                                         start=True, stop=True)
                        tmp = ostage.tile([128, 512], F32, tag="tmp", bufs=3)
                        if cc == 0:
                            nc.scalar.activation(tmp[:], p0c[:], AF.Copy,
                                                 scale=linv[:, tb4:tb4 + 1])
                        else:
                            nc.vector.tensor_scalar_mul(tmp[:], p0c[:],
                                                        linv[:, tb4:tb4 + 1])
                        nc.vector.scalar_tensor_tensor(
                            ot[:, bass.ts(cc, 512)], in0=p1c[:],
                            scalar=linv[:, 4 + tb4:4 + tb4 + 1], in1=tmp[:],
                            op0=ALU.mult, op1=ALU.add)
                    nc.sync.dma_start(
                        out=out_d[b * T + tgp:b * T + tgp + 128, :], in_=ot[:])


        for pool in (psum, ostage, lpool, pts, bigbufs, consts):
            pool.release()

    nc.compile()
    return nc


def _prep_inputs(x, W_qkv, b_qkv, W_proj, b_proj):
    """Host-side sharding/layout prep. Returns per-core in_maps."""
    bf16 = ml_dtypes.bfloat16
    x2 = np.ascontiguousarray(x.reshape(BT, C).T).astype(bf16)  # [C, B*T]
    scale = 1.0 / np.sqrt(D)

    jj, ii = np.meshgrid(np.arange(128), np.arange(128), indexing="ij")
    maskbf = np.where(jj <= ii, 0.0, MASK_VAL).astype(bf16)
    idbf = np.eye(128).astype(bf16)

    in_maps = []
    for core in range(NCORES):
        s = slice(128 * core, 128 * (core + 1))
        wq = (W_qkv[:, 0:C][:, s] * scale).astype(bf16)
        wk = W_qkv[:, C:2 * C][:, s].astype(bf16)
        wv = W_qkv[:, 2 * C:3 * C][:, s].astype(bf16)
        wp = W_proj[s, :].astype(bf16)
        bq = (b_qkv[0:C][s] * scale).astype(np.float32).reshape(128, 1)
        bk = b_qkv[C:2 * C][s].astype(np.float32).reshape(128, 1)
        bv = b_qkv[2 * C:3 * C][s].astype(np.float32)
        bvb = np.broadcast_to(bv, (128, 128)).copy()
        in_maps.append({
            "xt": x2, "wq": wq, "wk": wk, "wv": wv, "wp": wp,
            "bq": bq, "bk": bk, "bvb": bvb, "maskbf": maskbf, "idbf": idbf,
        })
    return in_maps


def kernel(x, W_qkv, b_qkv, W_proj, b_proj, _trace=False, _return_raw=False,
           _tmpdir=None):
    x = np.asarray(x, dtype=np.float32)
    W_qkv = np.asarray(W_qkv, dtype=np.float32)
    b_qkv = np.asarray(b_qkv, dtype=np.float32)
    W_proj = np.asarray(W_proj, dtype=np.float32)
    b_proj = np.asarray(b_proj, dtype=np.float32)

    if "nc" not in _compiled:
        _compiled["nc"] = _build()
    nc = _compiled["nc"]

    in_maps = _prep_inputs(x, W_qkv, b_qkv, W_proj, b_proj)
    kw = {}
    if _tmpdir is not None:
        kw["tmpdir"] = _tmpdir
    res = bass_utils.run_bass_kernel_spmd(
        nc, in_maps, core_ids=list(range(NCORES)), trace=_trace, **kw)

    acc = np.zeros((BT, C), dtype=np.float32)
    for core in range(NCORES):
        acc += res.results[core]["out"]
    acc += b_proj[None, :]
    out = acc.reshape(B, T, C)
    if _return_raw:
        return out, res
    return out


# revision 31
# speedup vs baseline: 1.1160x; 1.1160x over previous
"""Causal self-attention on 8 Trainium2 NeuronCores, head-sharded tensor parallel.

Contract: kernel(**inputs) takes the FULL unsharded inputs (x, W_qkv, b_qkv,
W_proj, b_proj) as numpy arrays and returns the FULL [B, T, C] float32 output.

Sharding: 16 heads / 8 cores = 2 heads per core. Each core computes qkv for
its heads, causal attention, and a partial output projection
(y_local @ W_proj[head_rows]); the host sums the 8 partials (the
tensor-parallel all-reduce, done at gather time) and adds b_proj.

Per-core kernel design (matmuls in bf16, fp32 accumulate):
- x is pre-transposed on host to xT [C, B*T] (bf16) so the contraction dim C
  is on SBUF partitions for the qkv matmuls.
- Q^T, K^T, V^T are produced in [d, t] layout (both heads packed into 128
  partitions, N=512 matmuls); V is then flipped to natural [t, d] layout by
  PE transposes, with a ones column appended per head ("V-aug") so the PV
  matmul also emits the softmax denominators l as psum row 64.
- Scores are computed transposed, S^T[j, i] = K Q^T, so exp(S^T) = P^T is
  born in the layout the PV matmul needs as rhs (no transposes of P).
- Softmax skips the max-subtraction: logits are ~N(0,1) by construction
  (1/sqrt(D) folded into Wq on host), max |logit| ~ 6, exp is safe in f32.
- Causal structure is exact at 128x128 tile granularity: above-diagonal
  tiles are never computed; diagonal tiles get an additive -87 mask.
- Y^T is normalized before eviction: l row -> columns (K=1 matmuls) ->
  reciprocal -> back to a row (K=128 matmul vs identity) -> gpsimd
  partition-broadcast -> one DVE multiply per head. The projection is then
  a single K=128 matmul per tile with a plain copy eviction.
"""
import sys

sys.path.insert(0, "/opt/trn_rl_repo")

import numpy as np
import ml_dtypes

import concourse.bacc as bacc
import concourse.bass as bass
import concourse.mybir as mybir
import concourse.tile as tile
from concourse import bass_utils

B, T, C, H, D = 2, 2048, 1024, 16, 64
NCORES = 8
HL = H // NCORES          # heads per core = 2
BT = B * T                # 4096
KT = C // 128             # 8 contraction tiles over C
NMC = BT // 1024          # 4 merged (1024-wide) column chunks over B*T
NTB = BT // 128           # 32 t-blocks of 128
NIC = T // 512            # 4 i-chunks per batch
BF16 = mybir.dt.bfloat16
F32 = mybir.dt.float32
AF = mybir.ActivationFunctionType
ALU = mybir.AluOpType
MASK_VAL = -87.0

_compiled = {}
_SKIP_LCHAIN = False
_SKIP_ATTN = False
_SKIP_PROJ = False
_SKIP_VT = False


def _build():
    nc = bacc.Bacc("TRN2", target_bir_lowering=False, debug=False)

    xt_d = nc.dram_tensor("xt", [C, BT], BF16, kind="ExternalInput")
    wq_d = nc.dram_tensor("wq", [C, 128], BF16, kind="ExternalInput")
    wk_d = nc.dram_tensor("wk", [C, 128], BF16, kind="ExternalInput")
    wv_d = nc.dram_tensor("wv", [C, 128], BF16, kind="ExternalInput")
    wp_d = nc.dram_tensor("wp", [128, C], BF16, kind="ExternalInput")
    bq_d = nc.dram_tensor("bq", [128, 1], F32, kind="ExternalInput")
    bk_d = nc.dram_tensor("bk", [128, 1], F32, kind="ExternalInput")
    bvb_d = nc.dram_tensor("bvb", [128, 128], F32, kind="ExternalInput")
    maskbf_d = nc.dram_tensor("maskbf", [128, 128], BF16, kind="ExternalInput")
    idbf_d = nc.dram_tensor("idbf", [128, 128], BF16, kind="ExternalInput")
    out_d = nc.dram_tensor("out", [BT, C], F32, kind="ExternalOutput")
    lsc = nc.dram_tensor("lsc", [B, NIC, 1024], F32)

    with tile.TileContext(nc) as tc:
        consts = tc.alloc_tile_pool(name="consts", bufs=1)
        bigbufs = tc.alloc_tile_pool(name="bigbufs", bufs=1)
        pts = tc.alloc_tile_pool(name="pts", bufs=3)
        lpool = tc.alloc_tile_pool(name="lpool", bufs=2)
        ostage = tc.alloc_tile_pool(name="ostage", bufs=4)
        psum = tc.alloc_tile_pool(name="psum", bufs=1, space="PSUM")

        def ps_s():
            return psum.tile([128, 2, 512], F32, tag="s", bufs=2, name="ps_s")

        # ---- constants ----
        wq_sb = consts.tile([128, KT, 128], BF16)
        wk_sb = consts.tile([128, KT, 128], BF16)
        wv_sb = consts.tile([128, KT, 128], BF16)
        for w_sb, w_d in ((wq_sb, wq_d), (wk_sb, wk_d), (wv_sb, wv_d)):
            nc.sync.dma_start(out=w_sb[:], in_=w_d.ap().rearrange("(k p) m -> p k m", p=128))
        wp_sb = consts.tile([128, C], BF16)
        nc.sync.dma_start(out=wp_sb[:], in_=wp_d[:, :])
        bq_sb = consts.tile([128, 1], F32)
        bk_sb = consts.tile([128, 1], F32)
        bvb_sb = consts.tile([128, 128], F32)
        maskbf_sb = consts.tile([128, 128], BF16)
        idbf_sb = consts.tile([128, 128], BF16)
        nc.sync.dma_start(out=bq_sb[:], in_=bq_d[:, :])
        nc.sync.dma_start(out=bk_sb[:], in_=bk_d[:, :])
        nc.sync.dma_start(out=bvb_sb[:], in_=bvb_d[:, :])
        nc.sync.dma_start(out=maskbf_sb[:], in_=maskbf_d[:, :])
        nc.sync.dma_start(out=idbf_sb[:], in_=idbf_d[:, :])

        # ---- persistent big buffers ----
        xt_sb = bigbufs.tile([128, KT, BT], BF16)       # 8 MB
        for g in range(NMC):
            for k in range(KT):
                nc.sync.dma_start(
                    out=xt_sb[:, k, bass.ts(g, 1024)],
                    in_=xt_d[k * 128:(k + 1) * 128, bass.ts(g, 1024)])
        qT = bigbufs.tile([128, BT], BF16)              # [2h*64 d, t]
        kTt = bigbufs.tile([128, BT], BF16)
        v_sb = bigbufs.tile([128, NTB, 130], BF16)      # [t, tb, 1|Vh0|Vh1|1]
        yt = bigbufs.tile([128, B, T], BF16)            # [2h*64 d, b, t] normalized

        nc.vector.memset(v_sb[:, :, 64:65], 1.0)
        nc.vector.memset(v_sb[:, :, 129:130], 1.0)

        # ---- stage A: Q^T, K^T (d-major, N=512 matmuls) ----
        for w_sb, b_sb, dst in ((wq_sb, bq_sb, qT), (wk_sb, bk_sb, kTt)):
            for mc in range(NMC):
                ps = ps_s()
                for k in range(KT):
                    for half in range(2):
                        nc.tensor.matmul(
                            ps[:, half, :], lhsT=w_sb[:, k, :],
                            rhs=xt_sb[:, k, mc * 1024 + half * 512:mc * 1024 + (half + 1) * 512],
                            start=(k == 0), stop=(k == KT - 1))
                nc.vector.tensor_scalar_add(
                    dst[:, bass.ts(mc, 1024)],
                    ps.rearrange("p h x -> p (h x)"), b_sb[:])

        # V in natural [t, d] layout directly: lhsT = xT tile, rhs = Wv
        for tb in range(NTB):
            pv = psum.tile([128, 128], F32, tag="y", bufs=2)
            for k in range(KT):
                nc.tensor.matmul(pv[:], lhsT=xt_sb[:, k, bass.ts(tb, 128)],
                                 rhs=wv_sb[:, k, :], start=(k == 0), stop=(k == KT - 1))
            nc.vector.tensor_add(v_sb[:, tb, 0:64], pv[:, 0:64], bvb_sb[:, 0:64])
            nc.vector.tensor_add(v_sb[:, tb, 65:129], pv[:, 64:128], bvb_sb[:, 64:128])

        # ---- stage B: attention + projection per batch ----
        for b in range(B):
            for ci in (range(NIC) if b == 0 else range(NIC - 1, -1, -1)):
                y0 = psum.tile([65, 512], F32, tag="y", bufs=2)
                y1 = psum.tile([65, 512], F32, tag="y", bufs=2)
                njb = 4 * (ci + 1)
                tg = b * T + ci * 512
                for jb in ([] if _SKIP_ATTN else range(njb)):
                    sb = max(0, jb - 4 * ci)
                    lo = sb * 128
                    jg = b * T + jb * 128
                    s2 = ps_s()
                    diag = jb >= 4 * ci
                    nc.tensor.matmul(s2[:, 0, lo:512], lhsT=kTt[0:64, jg:jg + 128],
                                     rhs=qT[0:64, tg + lo:tg + 512], start=True,
                                     stop=not diag)
                    nc.tensor.matmul(s2[:, 1, lo:512], lhsT=kTt[64:128, jg:jg + 128],
                                     rhs=qT[64:128, tg + lo:tg + 512], start=True,
                                     stop=not diag)
                    if diag:  # add the causal mask onto the diagonal 128 cols
                        nc.tensor.matmul(s2[:, 0, lo:lo + 128], lhsT=idbf_sb[:],
                                         rhs=maskbf_sb[:], start=False, stop=True)
                        nc.tensor.matmul(s2[:, 1, lo:lo + 128], lhsT=idbf_sb[:],
                                         rhs=maskbf_sb[:], start=False, stop=True)
                    ptb = pts.tile([128, 2, 512], BF16, tag="pt", bufs=4)
                    nc.scalar.activation(ptb[:, :, lo:512], s2[:, :, lo:512], AF.Exp)
                    vt = b * 16 + jb
                    nc.tensor.matmul(y0[:, lo:512], lhsT=v_sb[:, vt, 0:65],
                                     rhs=ptb[:, 0, lo:512], start=(jb == 0), stop=(jb == njb - 1))
                    nc.tensor.matmul(y1[:, lo:512], lhsT=v_sb[:, vt, 65:130],
                                     rhs=ptb[:, 1, lo:512], start=(jb == 0), stop=(jb == njb - 1))

                # evict unnormalized Y^T immediately (frees the y psums)
                nc.vector.tensor_copy(yt[0:64, b, ci * 512:(ci + 1) * 512],
                                      y0[0:64, :])
                nc.vector.tensor_copy(yt[64:128, b, ci * 512:(ci + 1) * 512],
                                      y1[0:64, :])
                # denominator rows -> columns (DRAM-bounced transpose) -> 1/l
                lr2 = lpool.tile([1, 1024], F32, tag="lr2", bufs=2)
                nc.scalar.copy(lr2[0:1, 0:512], y0[64:65, :])
                nc.scalar.copy(lr2[0:1, 512:1024], y1[64:65, :])
                nc.sync.dma_start(out=lsc[b, ci], in_=lr2[:])
                lcol = lpool.tile([128, 8], F32, tag="lcol", bufs=2)
                for h in range(HL):
                    nc.sync.dma_start(
                        out=lcol[:, h * 4:(h + 1) * 4],
                        in_=lsc[b, ci][h * 512:(h + 1) * 512].rearrange(
                            "(c p) -> p c", p=128))
                linv = lpool.tile([128, 8], F32, tag="linv", bufs=2)
                nc.vector.reciprocal(linv[:], lcol[:])

                # projection for this chunk's 4 t-blocks: per-head K=64
                # row-packed pairs, 1/l applied per head at eviction
                for tb4 in range(4):
                    tgp = ci * 512 + tb4 * 128
                    ot = ostage.tile([128, 1024], F32, tag="ot", bufs=4)
                    for cc in range(2):
                        p0c = psum.tile([128, 512], F32, tag="pj", bufs=2, name="p0c")
                        p1c = psum.tile([128, 512], F32, tag="pj", bufs=2, name="p1c")
                        nc.tensor.matmul(p0c[:], lhsT=yt[0:64, b, tgp:tgp + 128],
                                         rhs=wp_sb[0:64, bass.ts(cc, 512)],
                                         start=True, stop=True)
                        nc.tensor.matmul(p1c[:], lhsT=yt[64:128, b, tgp:tgp + 128],
                                         rhs=wp_sb[64:128, bass.ts(cc, 512)],
                                         start=True, stop=True)
                        tmp = ostage.tile([128, 512], F32, tag="tmp", bufs=3)
                        if cc == 0:
                            nc.scalar.activation(tmp[:], p0c[:], AF.Copy,
                                                 scale=linv[:, tb4:tb4 + 1])
                        else:
                            nc.vector.tensor_scalar_mul(tmp[:], p0c[:],
                                                        linv[:, tb4:tb4 + 1])
                        nc.vector.scalar_tensor_tensor(
                            ot[:, bass.ts(cc, 512)], in0=p1c[:],
                            scalar=linv[:, 4 + tb4:4 + tb4 + 1], in1=tmp[:],
                            op0=ALU.mult, op1=ALU.add)
                    nc.sync.dma_start(
                        out=out_d[b * T + tgp:b * T + tgp + 128, :], in_=ot[:])


        for pool in (psum, ostage, lpool, pts, bigbufs, consts):
            pool.release()

    nc.compile()
    return nc


def _prep_inputs(x, W_qkv, b_qkv, W_proj, b_proj):
    """Host-side sharding/layout prep. Returns per-core in_maps."""
    bf16 = ml_dtypes.bfloat16
    x2 = np.ascontiguousarray(x.reshape(BT, C).T).astype(bf16)  # [C, B*T]
    scale = 1.0 / np.sqrt(D)

    jj, ii = np.meshgrid(np.arange(128), np.arange(128), indexing="ij")
    maskbf = np.where(jj <= ii, 0.0, MASK_VAL).astype(bf16)
    idbf = np.eye(128).astype(bf16)

    in_maps = []
    for core in range(NCORES):
        s = slice(128 * core, 128 * (core + 1))
        wq = (W_qkv[:, 0:C][:, s] * scale).astype(bf16)
        wk = W_qkv[:, C:2 * C][:, s].astype(bf16)
        wv = W_qkv[:, 2 * C:3 * C][:, s].astype(bf16)
        wp = W_proj[s, :].astype(bf16)
        bq = (b_qkv[0:C][s] * scale).astype(np.float32).reshape(128, 1)
        bk = b_qkv[C:2 * C][s].astype(np.float32).reshape(128, 1)
        bv = b_qkv[2 * C:3 * C][s].astype(np.float32)
        bvb = np.broadcast_to(bv, (128, 128)).copy()
        in_maps.append({
            "xt": x2, "wq": wq, "wk": wk, "wv": wv, "wp": wp,
            "bq": bq, "bk": bk, "bvb": bvb, "maskbf": maskbf, "idbf": idbf,
        })
    return in_maps


def kernel(x, W_qkv, b_qkv, W_proj, b_proj, _trace=False, _return_raw=False,
           _tmpdir=None):
    x = np.asarray(x, dtype=np.float32)
    W_qkv = np.asarray(W_qkv, dtype=np.float32)
    b_qkv = np.asarray(b_qkv, dtype=np.float32)
    W_proj = np.asarray(W_proj, dtype=np.float32)
    b_proj = np.asarray(b_proj, dtype=np.float32)

    if "nc" not in _compiled:
        _compiled["nc"] = _build()
    nc = _compiled["nc"]

    in_maps = _prep_inputs(x, W_qkv, b_qkv, W_proj, b_proj)
    kw = {}
    if _tmpdir is not None:
        kw["tmpdir"] = _tmpdir
    res = bass_utils.run_bass_kernel_spmd(
        nc, in_maps, core_ids=list(range(NCORES)), trace=_trace, **kw)

    acc = np.zeros((BT, C), dtype=np.float32)
    for core in range(NCORES):
        acc += res.results[core]["out"]
    acc += b_proj[None, :]
    out = acc.reshape(B, T, C)
    if _return_raw:
        return out, res
    return out
